# revision 40
# baseline (speedup 1.0000x reference)
"""AttentionBlock kernel for Trainium2 (Bass/Tile), 8 NeuronCores.

Reference computation (B=4, C=256, H=W=64, Cqk=32, N=H*W=4096):
    q = Wq @ x + bq; k = Wk @ x + bk; v = Wv @ x + bv      (1x1 convs)
    energy[b,i,j] = sum_c q[b,c,i] k[b,c,j]
    attn = softmax(energy, axis=-1)
    out[b,c,i] = sum_j v[b,c,j] attn[b,i,j]
    result = gamma * out + x

Sharding: 8 cores = (batch b in 0..3) x (query-row half in 0..1).
Each core computes 2048 of the 4096 attention rows for one batch image;
the small conv weights are replicated.

Per-core layout choices:
  - energy is computed TRANSPOSED: energyT[j, i] with j on partitions.
    exp() is layout-agnostic; the softmax denominator s_i = sum_j exp()
    is obtained from a ones-column appended to vT in the attn@v matmul
    (outT[:, 256] = s_i), so no partition-axis reduction is ever needed.
  - No max-subtraction in softmax: |energy| <= ~45 for these scales, so
    exp() stays comfortably inside fp32 range; softmax ratios are exact.
  - The attn@v matmul produces outT[i, c] = sum_j expT[j,i] vT[j,c],
    normalized by gamma/s_i per partition, then PE-transposed back to
    [c, i] for the residual add with x.
  - PACK_E: the energy matmul has contraction Cqk=32, so 4 j-chunks run
    concurrently in the PE array as 32-row tiles (tile_position row
    packing). q/k are built 4x-replicated along partitions by tiling the
    projection weights host-side (zero extra device cost).

Measured state (v3, this session): TimelineSim single-shot span
119.8 -> 114.5 us; interleaved A/B vs the v2 baseline at R=16384 on HW:
median -38 us/iter (every pair negative) — the HW rewards the v3
dependency-graph changes far beyond the sim delta. HW-verified
scale-relative absmax error 8.5e-4.
Engine budget (cost model, per core): PE busy ~100 us (attn@v 512 mm,
energy 32 packed groups, fused k/q/v projections, 32 transposes — all
float32r), ACT exp 64 x 1.04 us + 16 osb scales = 74 us, DVE 29 us,
DMA 19.5 us.

v3 changes (all sim- and HW-verified together):
  - ONE flat software pipeline across all strips: the QD-deep queue
    carries over strip boundaries (energy/exp of strip s+1 overlap the
    drain + evac of strip s). PSUM stays legal because FIFO order means
    strip s+1's first attn@v only emits after strip s's last + evac,
    and the po pool rotation inserts the WAR hazards.
  - Per-strip out tiles (opool bufs=2) instead of one big out_sb:
    kills a per-strip tile-granularity WAR stall (output DMA read vs
    next strip's residual adds).
  - osb evac scale on ACT (AF.Copy, per-partition scale): Copy/Identity
    live in EVERY act table incl. exp's, so no table reload. Frees DVE
    and shortens the end-of-strip chain. AF.Reciprocal does NOT share a
    table with Exp — using it would thrash table loads.
  - gamma rides the host-side ones column (1/gamma instead of 1.0), so
    the denominator accumulates s/gamma and ONE DVE reciprocal per u
    yields gamma/s. PE transpose mode REQUIRES a pure 0/1 permutation
    RHS (CoreSim: "must be a permutation matrix"), so folding gamma
    into the transpose identity silently no-ops on HW — falsified.
  - Single-shot ramp: x loads as 256,256,512x7 subchunks with weight
    DMAs interleaved (wk, xc0a, wq, wv, xc0b, xc1..): first matmul at
    ~4.3 us (was ~6.5). HWDGE descriptors cost ~0.6 us SERIAL each and
    the DMA transfer stream is serialized at ~350 GB/s; both pace the
    ramp. (R-loop benching keeps weights outside the hardware loop.)
  - Pre-emitted strip-0 energy groups moved INSIDE the projection loop
    (each right after the k/q columns it needs, +512 cols of slack):
    ACT's exp pipeline starts ~8 us earlier, so ACT is no longer the
    straggler gating the final attn@v drain.
  - Output DMAs: one per strip, except the last strip split u0 / u1-2 /
    u3 (KERNEL_TAILDMA=1,3) to minimize serial-descriptor + last-
    transfer tail. Merged bias DMAs (bqk interleaved, bvg broadcast).
  - Final group of each strip emits u-major so po[0] completes 12
    matmuls before po[3] and the evac overlaps the last matmuls.
  - CP=258/257 (smaller vT pad) REJECTED by the neuronxcc backend
    (f32r free-dim %4) even though bass sim accepts it; CP=260 stands.
  - KERNEL_PE_SPLIT=0 (one 4-bank pe tile + single exp per group):
    sim +11 us — the 2-bank half-split's early PSUM release dominates
    the saved ACT fixed cost. Stays falsified.
  - Paired R-loop differencing CAVEAT: absolute us/iter INFLATES with
    R2 (134 at R2=4096-era baseline, ~168 at R2=16384 — p-state /
    thermal downclock under sustained load). Only interleaved A/B at
    the SAME R is trustworthy; absolute values are regime-dependent.

Key facts learned on this hardware (do not re-derive):
  - float32r with free-dim >= 256 streams 1 cyc/col — same as bf16; exact
    fp32 is 4 cyc/col. f32r is 4-byte fp32 storage; declaring the DRAM
    inputs + SBUF tiles f32r end-to-end needs no conversion copies (the
    BIR verifier accepts ExternalInputs as f32r producers, and DVE writes
    into f32r tiles perform the rounding). AP.bitcast(f32r) of an
    fp32-produced tile is REJECTED by the verifier.
  - walrus --enable-ldw-opt=true REJECTS bf16/16-bit InstLdweights
    ("not compatible with LDW optimization"), so bf16 matmuls would
    force ldw-opt off globally — a net loss. All-f32r is the optimum.
  - The hardware-loop bound (KERNEL_REPEAT) lowers to a runtime scalar:
    one NEFF serves every R (compile cache hits across R).
  - The axon tunnel adds ~1.2 s +-100 ms per execution; only paired
    (interleaved) differencing with R2 >= 4096 gives trustworthy times.

Falsified optimizations (do not retry without new evidence):
  - SW=256 strips w/ 2-bank energy buffers: matmul PSUM outputs at
    half-bank offsets crash the device (bank-align or don't).
  - 2-way packed half-groups (bufs=2): packing loss > overlap gain.
  - Half-group exps writing into ONE shared pe4 tile (old
    KERNEL_EXP_SPLIT=2): tile-granularity hazards keep the ping-pong, so
    only the ACT fixed cost lands (sim +5.6 us/iter). The WIN is
    KERNEL_PE_SPLIT=1 (now default): pe4 as two independent 2-bank tiles
    (bank-aligned, so no half-bank crash) + one exp per half — the next
    group's energy waits only on the first half-exp, ACT runs
    back-to-back (sim 137 -> 122 us; HW A/B median -4 us/iter, noisy).
  - KERNEL_VT_POOL=1 (vT bias-add on GPSIMD): Pool's software-efficiency
    penalty on the attn@v-critical path loses ~3 us (sim).
  - Moving wq/wv DMAs to the GPSIMD SWDGE queue to unclog the start ramp:
    SWDGE per-descriptor cost delivers the weights LATE and stalls q-proj
    (sim +5.5 us). The ~6 us single-shot start ramp is bound by the
    serialized ~350 GB/s DMA transfer stage, not by issue-queue order.
  - bf16 attn@v / osb: blocked by the ldw-opt incompatibility above.
PSUM is the binding constraint: pe4 halves (2+2 banks) + po[0..3]
(4 banks) fill all 8, which is why v-proj cannot interleave into the
strip loop and deeper energy double-buffering is impossible.

KERNEL_PRE_E=5 / KERNEL_QD=5 / KERNEL_EXPB=6 (defaults): 5 pre-emitted
strip-0 energy+exp groups feed a QD-deep software-pipeline queue with 6
exp buffers per half. v3 sim sweeps: QD=4..6 and EXPB 6..7 are all
within noise (the flat cross-strip queue removed the old sensitivity);
PRE_E=7 regresses (+6 us, exp-buffer pressure). Shipped config sim
span: 114.5 us; HW-verified rel_absmax 8.5e-4.

The NEFF compile cache is persistent on disk and keyed on the semantic
BIR (debug info stripped): a client-side compile_bass_kernel() smoke
test also warms the cache for the real run.
"""

import os

import numpy as np

B, C, H, W = 4, 256, 64, 64
CQK = 32
N = H * W                      # 4096
NCORES = 8
HALVES = 2                     # query-row halves per batch
NI = N // HALVES               # 2048 rows per core
P = 128                        # SBUF partitions
CC = C // P                    # 2 channel chunks
NJ = N // P                    # 32 key/value chunks
SW = 512                       # i-strip width
NSTRIP = NI // SW              # strips per core
PW = 512                       # projection tile width
NT_K = N // PW                 # k-proj tiles
NT_Q = NI // PW                # q-proj tiles
CP = int(os.environ.get("KERNEL_CP", "260"))  # vT width: 256 v-ch + ones + pad
G = 4                          # row-packing group size (128 / CQK)

# Defaults (v2): every matmul (energy, q/k/v projections, attn@v) runs in
# float32r — at free-dim >= 256 f32r streams 1 cyc/col like bf16 vs 4 for
# exact fp32, and x + conv weights are declared f32r end-to-end (f32r is
# 4-byte fp32 storage, so the host arrays pass through unchanged and no
# conversion copies exist). bf16 weights are NOT usable: walrus ldw-opt
# rejects bf16 InstLdweights ("not compatible with LDW optimization"),
# and dropping ldw-opt costs far more than bf16 would save (f32r already
# matches bf16 stream speed at these tile sizes).
_ENERGY_DT = os.environ.get("KERNEL_ENERGY_DT", "float32r")
_AV_DT = os.environ.get("KERNEL_AV_DT", "float32r")
_PACK_E = bool(int(os.environ.get("KERNEL_PACK_E", "1")))
_VPROJ_DT = os.environ.get("KERNEL_VPROJ_DT", "float32r")
_OSB_DT = os.environ.get("KERNEL_OSB_DT", "float32r")
# Compile walrus with --enable-ldw-opt=true: pipelines LDWEIGHTS under the
# previous matmul's stream (HW-measured 311 -> 153 ns per f32r matmul;
# output verified against the reference with the flag on).
_LDW_OPT = bool(int(os.environ.get("KERNEL_LDW_OPT", "1")))
# Software-pipeline the attn@v stage one group behind the energy/exp
# stage so exp(t) runs on ACT underneath group t-1's attn@v matmuls.
_SWP = bool(int(os.environ.get("KERNEL_SWP", "1")))
# Split the energy PSUM/exp into two 2-bank halves for finer PE/ACT overlap.
_PE_SPLIT = bool(int(os.environ.get("KERNEL_PE_SPLIT", "1")))
# Run the vT bias-add on GPSIMD (Pool) instead of DVE to unclog the
# projection-phase DVE critical path.
_VT_POOL = bool(int(os.environ.get("KERNEL_VT_POOL", "0")))
# Pre-emit this many strip-0 energy+exp groups inside the projection loop.
_PRE_E = int(os.environ.get("KERNEL_PRE_E", "5"))
_QD = int(os.environ.get("KERNEL_QD", "5"))
_EXPB = int(os.environ.get("KERNEL_EXPB", "6"))
_EVB = int(os.environ.get("KERNEL_EVB", "2"))
# Benchmark-only: repeat the computation R times in a hardware loop so
# device time dominates the (slow) tunnel round-trip.
_REPEAT = int(os.environ.get("KERNEL_REPEAT", "1"))

_CACHE = {}
LAST_RESULT = None


class _SplitView:
    """Present two [P, G//2, SW] tiles as one [P, G, SW] indexable."""

    def __init__(self, parts, per):
        self.parts = parts
        self.per = per

    def __getitem__(self, idx):
        _, g, sl = idx
        return self.parts[g // self.per][:, g % self.per, sl]

QKP = P if _PACK_E else CQK    # partition height of q/k tiles


def _enable_ldw_opt():
    """Recompile walrus flag --enable-ldw-opt=false -> true (in-process)."""
    import functools

    import concourse.bass_utils as bu

    if getattr(bu, "_ldw_opt_patched", False):
        return
    orig = bu.bir_verify_and_optimise

    @functools.wraps(orig)
    def patched(tmpdir, inp="bir.json", outp="file.neff", arch=None, *, dve_root=None):
        real = bu.run_command

        def hook(cmd, **kw):
            cmd = [
                "--enable-ldw-opt=true" if c == "--enable-ldw-opt=false" else c
                for c in cmd
            ]
            return real(cmd, **kw)

        bu.run_command = hook
        try:
            return orig(tmpdir, inp, outp, arch, dve_root=dve_root)
        finally:
            bu.run_command = real

    bu.bir_verify_and_optimise = patched
    bu._ldw_opt_patched = True


def _build_program():
    import contextlib

    if _LDW_OPT:
        _enable_ldw_opt()

    import concourse.bacc as bacc
    import concourse.bass as bass
    import concourse.mybir as mybir
    import concourse.tile as tile
    from concourse.bass import ts
    from concourse.masks import make_identity

    f32 = mybir.dt.float32
    e_dt = getattr(mybir.dt, _ENERGY_DT)
    av_dt = getattr(mybir.dt, _AV_DT)
    vp_dt = getattr(mybir.dt, _VPROJ_DT)
    osb_dt = getattr(mybir.dt, _OSB_DT)
    AF = mybir.ActivationFunctionType

    # f32r is 4-byte fp32 storage pre-rounded for the PE's fast path; the
    # BIR verifier requires f32r matmul operands to come from an f32r
    # producer. ExternalInputs declared f32r pass as-is (host fp32 data is
    # fine — the PE just truncates the low mantissa), and DVE writes into
    # f32r tiles perform the rounding. So x and the conv weights live in
    # pj_dt = f32r end-to-end and no conversion copies are needed.
    pj_dt = (
        f32 if (_ENERGY_DT == "float32" and _VPROJ_DT == "float32") else
        mybir.dt.float32r
    )

    nc = bacc.Bacc("TRN2", target_bir_lowering=False, debug=False)

    xb_d = nc.dram_tensor("xb", [C, N], pj_dt, kind="ExternalInput")
    wqT_d = nc.dram_tensor("wqT", [C, QKP], pj_dt, kind="ExternalInput")
    wkT_d = nc.dram_tensor("wkT", [C, QKP], pj_dt, kind="ExternalInput")
    wvT_d = nc.dram_tensor("wvT", [C, CP], pj_dt, kind="ExternalInput")
    bqk_d = nc.dram_tensor("bqk", [QKP * 2], f32, kind="ExternalInput")
    bvg_d = nc.dram_tensor("bvg", [CP + 1], f32, kind="ExternalInput")
    out_d = nc.dram_tensor("out", [C, NI], f32, kind="ExternalOutput")

    with tile.TileContext(nc) as tc:
        with (
            tc.tile_pool(name="consts", bufs=1) as consts,
            tc.tile_pool(name="sb", bufs=1) as sb,
            tc.tile_pool(name="xpool", bufs=2) as xpool,
            tc.tile_pool(name="opool", bufs=2) as opool,
            tc.tile_pool(name="evac", bufs=_EVB) as evac,
            tc.tile_pool(name="expp", bufs=_EXPB) as expp,
            tc.tile_pool(name="psE", bufs=1 if _PACK_E else 2, space="PSUM") as psE,
            tc.tile_pool(name="psO", bufs=4, space="PSUM") as psO,
        ):
            ctx_psM = (
                contextlib.nullcontext()
                if _PACK_E
                else tc.tile_pool(name="psM", bufs=2, space="PSUM")
            )
            with ctx_psM as psM:
                # ---- constants / weights ----
                ident = consts.tile([P, P], f32)
                make_identity(nc, ident[:, :])

                wq_sb = consts.tile([P, CC, QKP], pj_dt)
                wk_sb = consts.tile([P, CC, QKP], pj_dt)
                wv_sb = consts.tile([P, CC, CP], pj_dt)

                def load_wq():
                    nc.sync.dma_start(
                        out=wq_sb[:, :, :],
                        in_=wqT_d.ap().rearrange("(cc p) o -> p cc o", p=P),
                    )

                def load_wk():
                    nc.sync.dma_start(
                        out=wk_sb[:, :, :],
                        in_=wkT_d.ap().rearrange("(cc p) o -> p cc o", p=P),
                    )

                def load_wv():
                    nc.sync.dma_start(
                        out=wv_sb[:, :, :],
                        in_=wvT_d.ap().rearrange("(cc p) c -> p cc c", p=P),
                    )

                # Weight DMAs each occupy a serial ~0.6us HWDGE descriptor
                # slot plus transfer time ahead of the x chunks, delaying
                # xc[0] and with it the whole PE start. Single-shot: issue
                # xc[0] FIRST, then wk/wq (needed ~2 matmuls in), then wv
                # (first v-proj). R-loop benching keeps weights outside the
                # hardware loop (an instruction after the loop could never
                # feed iteration 1).
                if _REPEAT > 1:
                    load_wq()
                    load_wk()
                    load_wv()

                # Two merged SWDGE loads instead of four: bq/bk interleaved
                # host-side into one [QKP, 2] tile; bv (+ trailing 1.0 ones
                # column) and gamma concatenated into one partition-
                # broadcast [P, CP + 1] tile.
                bqk_sb = consts.tile([QKP, 2], f32)
                nc.gpsimd.dma_start(
                    out=bqk_sb[:, :], in_=bass.AP(bqk_d, 0, [[2, QKP], [1, 2]])
                )
                bq_sb = bqk_sb[:, 0:1]
                bk_sb = bqk_sb[:, 1:2]
                bvg_sb = consts.tile([P, CP + 1], f32)
                nc.gpsimd.dma_start(
                    out=bvg_sb[:, :],
                    in_=bass.AP(bvg_d, 0, [[0, P], [1, CP + 1]]),
                )
                bvb_sb = bvg_sb[:, 0:CP]
                # gamma enters via the host-prepared ones column: vT's
                # denominator column is 1/gamma, so po[:, C] accumulates
                # s/gamma and the single reciprocal yields gamma/s — no
                # extra device op, no gamma-scaled identity (PE transpose
                # mode requires a pure 0/1 permutation as its RHS).
                ident_g = consts.tile([P, P], osb_dt)
                nc.vector.tensor_copy(ident_g[:, :], ident[:, :])

                rep = (
                    tc.For_i(0, _REPEAT, 1)
                    if _REPEAT > 1
                    else contextlib.nullcontext()
                )
                with rep:
                    # ---- activations ----
                    # x arrives column-rotated so this core's 2048 query
                    # columns are always cols 0:NI (attention is permutation-
                    # invariant over key/value positions, so rotating the key
                    # axis changes nothing). x is loaded as 8 SEPARATE
                    # 512-column tiles in k-proj consumption order: distinct
                    # tiles make the DMA->matmul hazards per-chunk, so the
                    # first k-proj matmul starts after ~1/8 of the transfer
                    # instead of waiting for the whole 4MB. Double-buffered
                    # (xpool bufs=2) so the next hardware-loop iteration's
                    # load overlaps this iteration's attention strips.
                    xb_src = xb_d.ap().rearrange("(cc p) n -> p cc n", p=P)
                    # Subchunk widths: two 256-col leading chunks (single
                    # shot) so the first k-proj matmul waits for ~0.7us of
                    # transfer instead of 1.46us; weight DMAs interleave so
                    # each lands just before its first consumer.
                    if _REPEAT == 1:
                        prefix = [
                            int(w)
                            for w in os.environ.get(
                                "KERNEL_WIDTHS", "256,256"
                            ).split(",")
                            if w
                        ]
                        widths = prefix + [PW] * ((N - sum(prefix)) // PW)
                        load_wk()
                    else:
                        widths = [PW] * (N // PW)
                    xcs = []           # (tile, col0, width)
                    col0 = 0
                    for ci, w_ in enumerate(widths):
                        t_ = xpool.tile([P, CC, w_], pj_dt, tag=f"xc{ci}")
                        nc.sync.dma_start(
                            out=t_[:, :, :],
                            in_=xb_src[:, :, col0 : col0 + w_],
                        )
                        xcs.append((t_, col0, w_))
                        col0 += w_
                        _WV_AT = int(os.environ.get("KERNEL_WV_AT", "0"))
                        if _REPEAT == 1 and ci == 0:
                            load_wq()
                            if _WV_AT == 0:
                                load_wv()
                        if _REPEAT == 1 and ci == _WV_AT and _WV_AT > 0:
                            load_wv()

                    def xq_at(i0):
                        """(tile, local col) covering cols [i0, i0+P)."""
                        for t_, c0, w_ in xcs:
                            if c0 <= i0 < c0 + w_:
                                return t_, i0 - c0
                        raise AssertionError(i0)

                    q_sb = sb.tile([QKP, NI], e_dt)
                    k_sb = sb.tile([QKP, N], e_dt)
                    vt_sb = sb.tile([P, NJ, CP], av_dt)

                    def emit_eg(s, t):
                        """Energy group t of strip s + exp; returns the ex
                        view for the attn@v stage (PACK_E layouts)."""
                        if _PE_SPLIT:
                            # pe4 as two independent 2-bank tiles: the next
                            # group's g=0,1 energy matmuls only wait for
                            # the first half-exp, so ACT runs back-to-back
                            # instead of the exp -> energy -> exp ping-pong.
                            pes = [
                                psE.tile(
                                    [P, G // 2, SW], f32, tag="pea", name="pea"
                                ),
                                psE.tile(
                                    [P, G // 2, SW], f32, tag="peb", name="peb"
                                ),
                            ]
                            exs = [
                                expp.tile(
                                    [P, G // 2, SW], av_dt, tag="exa",
                                    name="exa",
                                ),
                                expp.tile(
                                    [P, G // 2, SW], av_dt, tag="exb",
                                    name="exb",
                                ),
                            ]
                            for h in range(2):
                                for gg in range(G // 2):
                                    g = h * (G // 2) + gg
                                    j = G * t + g
                                    nc.tensor.matmul(
                                        pes[h][:, gg, :],
                                        k_sb[32 * g : 32 * (g + 1), ts(j, P)],
                                        q_sb[
                                            32 * g : 32 * (g + 1), ts(s, SW)
                                        ],
                                        start=True,
                                        stop=True,
                                        tile_position=(32 * g, 0),
                                    )
                                nc.scalar.activation(
                                    exs[h][:, :, :], pes[h][:, :, :], AF.Exp
                                )
                            return _SplitView(exs, G // 2)
                        pe4 = psE.tile([P, G, SW], f32, tag="pe")
                        for g in range(G):
                            j = G * t + g
                            nc.tensor.matmul(
                                pe4[:, g, :],
                                k_sb[32 * g : 32 * (g + 1), ts(j, P)],
                                q_sb[32 * g : 32 * (g + 1), ts(s, SW)],
                                start=True,
                                stop=True,
                                tile_position=(32 * g, 0),
                            )
                        ex4 = expp.tile([P, G, SW], av_dt, tag="ex")
                        nc.scalar.activation(ex4[:, :, :], pe4[:, :, :], AF.Exp)
                        return ex4

                    # Pre-emit strip-0 energy+exp groups INSIDE the
                    # projection loop, each as soon as the k/q columns it
                    # reads are projected: ACT starts its exp pipeline
                    # ~8us earlier (right after the second x subchunk)
                    # instead of idling until all projections are queued,
                    # so it is no longer the straggler gating the final
                    # attn@v drain.
                    pre_q = []

                    _PRE_SLACK = int(os.environ.get("KERNEL_PRE_SLACK", "512"))

                    def pre_emit(end_col, final=False):
                        if not (_PACK_E and _SWP):
                            return
                        while len(pre_q) < _PRE_E and (
                            final
                            or (
                                end_col
                                >= (len(pre_q) + 1) * G * P + _PRE_SLACK
                                and end_col >= SW
                            )
                        ):
                            t = len(pre_q)
                            pre_q.append((emit_eg(0, t), t, 0))

                    # ---- projections ----
                    # One fused pass per x subchunk: k-tile, q-tile, then
                    # the vT j-chunks that read the same x subchunk. Each
                    # block consumes the subchunk the k-tile just waited
                    # for, so the PE tracks the streaming x DMA instead of
                    # idling through it.
                    # k = Wk @ xb + bk (PACK_E: 4x-replicated on partitions)
                    for ci, (xt, c0, w_) in enumerate(xcs):
                        ps = psO.tile([QKP, w_], f32, tag="po", name=f"psk{ci}")
                        for cc in range(CC):
                            nc.tensor.matmul(
                                ps[:, :],
                                wk_sb[:, cc, :],
                                xt[:, cc, :],
                                start=(cc == 0),
                                stop=(cc == CC - 1),
                            )
                        nc.vector.tensor_scalar_add(
                            k_sb[:, c0 : c0 + w_], ps[:, :], bk_sb
                        )
                        # q = Wq @ xq + bq (query cols are cols 0..NI-1)
                        if c0 < NI:
                            ps = psO.tile(
                                [QKP, w_], f32, tag="po", name=f"psq{ci}"
                            )
                            for cc in range(CC):
                                nc.tensor.matmul(
                                    ps[:, :],
                                    wq_sb[:, cc, :],
                                    xt[:, cc, :],
                                    start=(cc == 0),
                                    stop=(cc == CC - 1),
                                )
                            nc.vector.tensor_scalar_add(
                                q_sb[:, c0 : c0 + w_], ps[:, :], bq_sb
                            )
                        # vT = (Wv @ xb + bv).T; wvT's zero columns plus
                        # bv's trailing 1.0 produce the ones column that
                        # yields the softmax denominator in attn@v.
                        for j in range(c0 // P, (c0 + w_) // P):
                            ps = psO.tile([P, CP], f32, tag="po", name=f"psv{j}")
                            for cc in range(CC):
                                nc.tensor.matmul(
                                    ps[:, :],
                                    xt[:, cc, j * P - c0 : (j + 1) * P - c0],
                                    wv_sb[:, cc, :],
                                    start=(cc == 0),
                                    stop=(cc == CC - 1),
                                )
                            veng = nc.gpsimd if _VT_POOL else nc.vector
                            veng.tensor_add(
                                vt_sb[:, j, :], ps[:, :], bvb_sb[:, :]
                            )
                        pre_emit(c0 + w_)
                    pre_emit(N, final=True)

                    # ---- attention strips ----
                    po_of = {}

                    def get_po(s_p):
                        if s_p not in po_of:
                            po_of[s_p] = [
                                psO.tile(
                                    [P, CP], f32, tag="po",
                                    name=f"po{s_p}_{u}",
                                )
                                for u in range(SW // P)
                            ]
                        return po_of[s_p]

                    def emit_o(ex4_p, t_p, s_p):
                        po = get_po(s_p)
                        # Final group of a strip: u-major order, so po[0]
                        # finishes 12 matmuls before po[3] and its evac
                        # chain (osb/transpose/add/DMA) overlaps the last
                        # matmuls instead of all four starting at the end.
                        if t_p == NJ // G - 1:
                            order = [
                                (g, u)
                                for u in range(SW // P)
                                for g in range(G)
                            ]
                        else:
                            order = [
                                (g, u)
                                for g in range(G)
                                for u in range(SW // P)
                            ]
                        for g, u in order:
                            j = G * t_p + g
                            nc.tensor.matmul(
                                po[u][:, :],
                                ex4_p[:, g, ts(u, P)],
                                vt_sb[:, j, :],
                                start=(t_p == 0 and g == 0),
                                stop=(t_p == NJ // G - 1 and g == G - 1),
                            )

                    def evac_strip(s):
                        # Evac in two passes: first drain every po[u] into
                        # SBUF (frees all four PSUM banks for the next
                        # strip's attn@v as early as possible), then the
                        # transpose/residual/DMA chain.
                        po = po_of.pop(s)
                        osbs = []
                        for u in range(SW // P):
                            r = evac.tile([P, 1], f32, tag="r")
                            nc.vector.reciprocal(r[:, :], po[u][:, C : C + 1])
                            osb = evac.tile([P, C], osb_dt, tag=f"osb{u}")
                            # osb = po / s on ACT: Copy is in every act
                            # table (incl. exp's) so no table reload; frees
                            # DVE and shortens the end-of-strip chain (ACT
                            # is idle once the strip's exps are done). The
                            # gamma factor rides the transpose identity.
                            nc.scalar.activation(
                                osb[:, :], po[u][:, 0:C], AF.Copy,
                                scale=r[:, :],
                            )
                            osbs.append(osb)
                        # Per-strip output tile (bufs=2) so the output DMA's
                        # read and the next strip's residual adds touch
                        # different buffers (no tile-granularity WAR stall).
                        out_t = opool.tile([P, CC, SW], f32, tag="out")
                        out_ap = out_d.ap().rearrange("(cc p) n -> p cc n", p=P)
                        for u in range(SW // P):
                            i0 = s * SW + u * P
                            osb = osbs[u]
                            for ch in range(CC):
                                pool = psO if _PACK_E else psM
                                pt = pool.tile(
                                    [P, P],
                                    osb_dt,
                                    tag="po" if _PACK_E else "ps",
                                    name=f"pt{s}_{u}_{ch}",
                                )
                                nc.tensor.transpose(
                                    pt[:, :], osb[:, ts(ch, P)], ident_g[:, :]
                                )
                                xt, xo = xq_at(i0)
                                nc.vector.tensor_add(
                                    out_t[:, ch, u * P : u * P + P],
                                    pt[:, :],
                                    xt[:, ch, xo : xo + P],
                                )
                            # Last strip: split the output DMA so the final
                            # descriptor+transfer chain after the last evac
                            # is short. HWDGE descriptors cost ~0.6us
                            # SERIAL, more than the 0.36us transfer, so the
                            # split pattern matters; modes are cumulative
                            # u-boundaries, e.g. "3" = u0-2 then u3.
                            if s == NSTRIP - 1:
                                bounds = [
                                    int(b)
                                    for b in os.environ.get(
                                        "KERNEL_TAILDMA", "1,3"
                                    ).split(",")
                                    if b
                                ] + [SW // P]
                                if u + 1 in bounds:
                                    lo = 0
                                    for b in bounds:
                                        if b == u + 1:
                                            break
                                        lo = b
                                    nc.sync.dma_start(
                                        out=out_ap[
                                            :, :,
                                            s * SW + lo * P : i0 + P,
                                        ],
                                        in_=out_t[:, :, lo * P : u * P + P],
                                    )
                        if s < NSTRIP - 1:
                            # One strip-wide DMA: fewer HWDGE descriptors.
                            nc.sync.dma_start(
                                out=out_ap[:, :, ts(s, SW)],
                                in_=out_t[:, :, :],
                            )

                    if _PACK_E and _SWP:
                        # One flat software pipeline across ALL strips: the
                        # queue carries over strip boundaries so ACT keeps
                        # computing the next strip's exps while the previous
                        # strip's attn@v drains + evacs (the old per-strip
                        # drain stalled PE ~0.45us at every boundary). PSUM
                        # stays legal: strip s+1's first emit_o only happens
                        # after strip s's last emit_o + evac (FIFO order),
                        # and the po pool rotation inserts the WAR hazards.
                        queue = list(pre_q)
                        groups = [
                            (s, t)
                            for s in range(NSTRIP)
                            for t in range(NJ // G)
                        ][len(pre_q):]
                        for s, t in groups:
                            if len(queue) >= _QD:
                                ex_p, t_p, s_p = queue.pop(0)
                                emit_o(ex_p, t_p, s_p)
                                if t_p == NJ // G - 1:
                                    evac_strip(s_p)
                            queue.append((emit_eg(s, t), t, s))
                        for ex_p, t_p, s_p in queue:
                            emit_o(ex_p, t_p, s_p)
                            if t_p == NJ // G - 1:
                                evac_strip(s_p)
                    elif _PACK_E:
                        for s in range(NSTRIP):
                            for t in range(NJ // G):
                                emit_o(emit_eg(s, t), t, s)
                            evac_strip(s)
                    else:
                        for s in range(NSTRIP):
                            po = get_po(s)
                            for j in range(NJ):
                                pe = psE.tile([P, SW], f32, tag="pe")
                                nc.tensor.matmul(
                                    pe[:, :],
                                    k_sb[:, ts(j, P)],
                                    q_sb[:, ts(s, SW)],
                                    start=True,
                                    stop=True,
                                )
                                ex = expp.tile([P, SW], av_dt, tag="ex")
                                nc.scalar.activation(ex[:, :], pe[:, :], AF.Exp)
                                for u in range(SW // P):
                                    nc.tensor.matmul(
                                        po[u][:, :],
                                        ex[:, ts(u, P)],
                                        vt_sb[:, j, :],
                                        start=(j == 0),
                                        stop=(j == NJ - 1),
                                    )
                            evac_strip(s)

    nc.compile()
    return nc


def _host_prep(inputs):
    """Common host-side input preparation for all variants."""
    x = np.ascontiguousarray(np.asarray(inputs["x"], dtype=np.float32))
    Wq = np.asarray(inputs["Wq"], dtype=np.float32)
    Wk = np.asarray(inputs["Wk"], dtype=np.float32)
    Wv = np.asarray(inputs["Wv"], dtype=np.float32)
    bq = np.ascontiguousarray(np.asarray(inputs["bq"], dtype=np.float32))
    bk = np.ascontiguousarray(np.asarray(inputs["bk"], dtype=np.float32))
    bv = np.ascontiguousarray(np.asarray(inputs["bv"], dtype=np.float32))
    gamma = np.ascontiguousarray(np.asarray(inputs["gamma"], dtype=np.float32))

    xf = x.reshape(B, C, N)
    wqT = np.ascontiguousarray(Wq.T)
    wkT = np.ascontiguousarray(Wk.T)
    if _PACK_E:
        wqT = np.ascontiguousarray(np.tile(wqT, (1, G)))
        wkT = np.ascontiguousarray(np.tile(wkT, (1, G)))
        bq = np.ascontiguousarray(np.tile(bq, G))
        bk = np.ascontiguousarray(np.tile(bk, G))
    wvT = np.ascontiguousarray(
        np.concatenate([Wv.T, np.zeros((C, CP - C), np.float32)], axis=1)
    )
    bvp = np.concatenate(
        [bv, np.ones((1,), np.float32), np.zeros((CP - C - 1,), np.float32)]
    )
    # Merged bias tensors: bq/bk interleaved per-partition; bv + 1/gamma
    # ones column (so the accumulated denominator is s/gamma and a single
    # on-device reciprocal yields gamma/s; gamma == 0 gives r = 0 and the
    # output degrades to x, matching the reference). The trailing slot is
    # unused padding now.
    bqk = np.ascontiguousarray(np.stack([bq, bk], axis=1).ravel())
    with np.errstate(divide="ignore"):
        ginv = np.float32(1.0) / gamma[:1].astype(np.float32)
    bvp = bvp.copy()
    bvp[C] = ginv[0]
    bvg = np.ascontiguousarray(
        np.concatenate([bvp, np.zeros((1,), np.float32)])
    )

    in_maps = []
    for core in range(NCORES):
        b, half = divmod(core, HALVES)
        sl = slice(half * NI, (half + 1) * NI)
        in_maps.append(
            {
                "xb": np.ascontiguousarray(np.roll(xf[b], -half * NI, axis=1)),
                "wqT": wqT,
                "wkT": wkT,
                "wvT": wvT,
                "bqk": bqk,
                "bvg": bvg,
            }
        )
    return in_maps


def kernel(**inputs):
    global LAST_RESULT
    from concourse.bass_utils import run_bass_kernel_spmd

    if "nc" not in _CACHE:
        _CACHE["nc"] = _build_program()
    nc = _CACHE["nc"]

    in_maps = _host_prep(inputs)

    trace = bool(os.environ.get("KERNEL_TRACE"))
    kwargs = {}
    if trace and os.environ.get("KERNEL_TRACE_ALL"):
        kwargs["trace_cores"] = list(range(NCORES))
        kwargs["stitch_traces"] = True
    res = run_bass_kernel_spmd(
        nc, in_maps, core_ids=list(range(NCORES)), trace=trace, **kwargs
    )
    LAST_RESULT = res

    out = np.empty((B, C, N), dtype=np.float32)
    for core in range(NCORES):
        b, half = divmod(core, HALVES)
        out[b][:, half * NI : (half + 1) * NI] = res.results[core]["out"]
    return out.reshape(B, C, H, W)

